# revision 4
# baseline (speedup 1.0000x reference)
"""MoE routing kernel v2 for Trainium2 (8 NeuronCores).

Same math as v1 (see kernel.py docstring): the identity-activation MoE +
final sum over D collapses to per-expert scalars
    sum_d o[n,e,d] = x[n] . v[e] + c[e],   v[e] = w1[e] @ rowsum(w2[e])
and top-2 gating reduces to w0 = sigmoid(l0 - l1).

v2 changes vs v1:
  * Host-side layout prep: w1/w2 are cast to bf16 (halves their HBM
    traffic; only the smooth q = x.v + c path sees them, error ~1e-1
    abs << the 1.9 abs gate) and x is pre-transposed per batch row
    (xT: [D, T], fp32) so the gating matmul needs NO on-device
    transposes of x. Logits stay fp32 end-to-end: top-2 *selection* is
    discontinuous in the runner-up, so bf16 logits flip near-ties and
    cost O(1-10) errors.
  * One collective per rep instead of two: cores 2b, 2b+1 both gate the
    FULL batch row b (1024 tokens) redundantly, so the log-softmax
    denominator is local and the s AllGather disappears. Each core
    writes its own half, selected by an `hsel` input.
  * vc packed p-major (vc[p*9+j]) so both the publish DMA and the
    post-collective gather are 128/1024-descriptor DMAs, not 4-byte
    element scatters.
  * Gating is batched across the 8 token-chunks (3D APs) instead of
    per-chunk; the c[e] bias folds into the PSUM->SBUF copy as a
    per-partition tensor_scalar_add.
  * SBUF-local log-softmax tail (PE dot/broadcast tricks) - no DRAM
    round-trips.
"""

import numpy as np
import ml_dtypes

import concourse.bass as bass
import concourse.tile as tile
from concourse import mybir
from concourse.bass_utils import run_bass_kernel_spmd
from concourse.masks import make_identity

B, T, D, H, E = 4, 1024, 1024, 2048, 8
N = B * T
NCORES = 8
NTOK = N // NCORES      # 512 tokens per core (output shard)
F32 = mybir.dt.float32
BF16 = mybir.dt.bfloat16
NPBF16 = ml_dtypes.bfloat16
AX = mybir.AxisListType
OP = mybir.AluOpType
ACTF = mybir.ActivationFunctionType

ND = D // 128    # 8
NH = H // 128    # 16
NC = T // 128    # 8 token chunks per batch row
VC2 = 128 * (ND + 1)  # 1152, vc[p*9+j]: j<8 -> v[j*128+p], j=8 -> c

_CACHE = {}


def _legalize_waits(nc):
    """Walrus accepts only one sync-wait slot on most TRN2 instruction
    encodings. Move surplus waits onto an InstDrain inserted immediately
    before the offender on the same engine (drains accept many waits -- the
    Tile tail barrier relies on that). Same-engine order is preserved, so
    semantics are unchanged."""
    # EVENT_SEMAPHORE_RANGE_CLEAR (isa opcode 176) crashes this runtime
    # (NRT_EXEC_UNIT_UNRECOVERABLE); the is_reset_sema drain already resets
    # the tile sems, and the barrier butterfly leaves its event sems at 0,
    # so dropping it is safe (verified over repeated executions).
    for bb in nc.main_func.blocks:
        bb.instructions = [i for i in bb.instructions
                           if "EVENT_SEMAPHORE_RANGE_CLEAR" not in str(i)]
    offenders = []
    for bb in nc.main_func.blocks:
        for inst in bb.instructions:
            si = inst.sync_info
            if si is None:
                continue
            if len(si.on_wait) > 1:
                offenders.append((bb, inst))
    import bass_rust as _br
    for bb, inst in offenders:
        si = inst.sync_info
        waits = list(si.on_wait)
        si.on_wait = [waits[-1]]
        idx = bb.instructions.index(inst)
        for w in reversed(waits[:-1]):
            d = nc.engines[inst.engine].nop(nofuse=True, hint="wait_legalize")
            dins = d.ins
            for bb2 in nc.main_func.blocks:
                if dins in bb2.instructions:
                    bb2.instructions.remove(dins)
            dins.sync_info = _br.SyncInfo(on_wait=[w], on_update=[])
            bb.instructions.insert(idx, dins)


def _build_nc(reps: int = 1, variant: str = "full") -> bass.Bass:
    nc = bass.Bass("TRN2", target_bir_lowering=False)

    w1e = nc.dram_tensor("w1e", [D, H], BF16, kind="ExternalInput")
    w2e = nc.dram_tensor("w2e", [H, D], BF16, kind="ExternalInput")
    b1c = nc.dram_tensor("b1c", [128, NH], F32, kind="ExternalInput")
    b2c = nc.dram_tensor("b2c", [128, ND], F32, kind="ExternalInput")
    xTr = nc.dram_tensor("xTr", [D, T], F32, kind="ExternalInput")
    wgl = nc.dram_tensor("wgl", [128, ND * E], F32, kind="ExternalInput")
    hsel = nc.dram_tensor("hsel", [1, 1], F32, kind="ExternalInput")
    yout = nc.dram_tensor("y", [NTOK], F32, kind="ExternalOutput")

    vc_in = nc.dram_tensor("vc_in", [VC2], F32)
    vc_all = nc.dram_tensor("vc_all", [NCORES, VC2], F32, addr_space="Shared")

    RG = [list(range(NCORES))]

    use_coll = variant in ("full", "collonly", "phaseB")
    do_phaseA = variant in ("full", "nocoll", "phaseA")
    do_phaseB = variant in ("full", "nocoll", "phaseB")

    with tile.TileContext(nc) as tc:
      with tc.tile_pool(name="consts", bufs=1) as consts:
        identF = consts.tile([128, 128], F32)
        make_identity(nc, identF)
        ones_col = consts.tile([128, 1], F32)
        nc.vector.memset(ones_col, 1.0)
        ones_row = consts.tile([1, 128], F32)
        nc.vector.memset(ones_row, 1.0)
        negones_row = consts.tile([1, 128], F32)
        nc.vector.memset(negones_row, -1.0)
        for _rep in range(reps):
          with (
            tc.tile_pool(name="singles", bufs=1) as singles,
            tc.tile_pool(name="w2pool", bufs=16) as w2pool,
            tc.tile_pool(name="w1pool", bufs=8) as w1pool,
            tc.tile_pool(name="xpool", bufs=8) as xpool,
            tc.tile_pool(name="gpool", bufs=2) as gpool,
            tc.tile_pool(name="spool", bufs=2) as spool,
            tc.tile_pool(name="psO", bufs=2, space="PSUM") as psO,
            tc.tile_pool(name="psG", bufs=1, space="PSUM") as psG,
            tc.tile_pool(name="psS", bufs=2, space="PSUM") as psS,
        ):
            if variant == "stub":
                ydummy = spool.tile([1, NTOK], F32)
                nc.vector.memset(ydummy, 0.0)
                nc.gpsimd.dma_start(out=yout.ap(), in_=ydummy)
                continue

            v_cols = singles.tile([128, ND + 1], F32)

            if do_phaseA:
                # -------- Phase A: expert-local v = w1 @ rowsum(w2) --------
                w2s_cols = singles.tile([128, NH], F32)
                for j in range(NH):
                    w2t = w2pool.tile([128, D], BF16)
                    nc.sync.dma_start(out=w2t, in_=w2e[j * 128:(j + 1) * 128, :])
                    nc.vector.tensor_reduce(
                        out=w2s_cols[:, j:j + 1], in_=w2t, axis=AX.X, op=OP.add
                    )

                # w2s broadcast to all partitions via a DRAM bounce (cast bf16)
                w2s_dram = nc.dram_tensor(f"w2s_dram_{_rep}", [1, H], F32)
                nc.gpsimd.dma_start(
                    out=w2s_dram.ap().rearrange("one (j p) -> p (one j)", p=128),
                    in_=w2s_cols,
                )
                w2s_b = singles.tile([128, H], BF16)
                nc.gpsimd.dma_start(
                    out=w2s_b, in_=w2s_dram.ap().to_broadcast((128, H))
                )

                # v[d] = sum_h w1[d, h] * w2s[h] (fused mul+reduce per d-chunk;
                # TTR miscompiles on this walrus build, STT+accum_out doesn't)
                for jd in range(ND):
                    w1t = w1pool.tile([128, H], BF16)
                    nc.sync.dma_start(out=w1t, in_=w1e[jd * 128:(jd + 1) * 128, :])
                    nc.vector.scalar_tensor_tensor(
                        out=w1t, in0=w1t, scalar=1.0, in1=w2s_b,
                        op0=OP.mult, op1=OP.mult,
                        accum_out=v_cols[:, jd:jd + 1],
                    )

                # c = b1 . w2s + sum(b2), all lanes then one PE dot
                b1sb = singles.tile([128, NH], F32)
                nc.sync.dma_start(out=b1sb, in_=b1c[:, :])
                b2sb = singles.tile([128, ND], F32)
                nc.sync.dma_start(out=b2sb, in_=b2c[:, :])
                scrA = singles.tile([128, NH], F32)
                cpart = singles.tile([128, 1], F32)
                nc.vector.scalar_tensor_tensor(
                    out=scrA, in0=b1sb, scalar=1.0, in1=w2s_cols,
                    op0=OP.mult, op1=OP.mult, accum_out=cpart,
                )
                b2part = singles.tile([128, 1], F32)
                nc.vector.tensor_reduce(out=b2part, in_=b2sb, axis=AX.X, op=OP.add)
                ctot = singles.tile([128, 1], F32)
                nc.vector.tensor_add(out=ctot, in0=cpart, in1=b2part)
                pc = psS.tile([1, 1], F32, tag="pss")
                nc.tensor.matmul(pc, lhsT=ctot, rhs=ones_col, start=True, stop=True)
                c_sb = singles.tile([1, 1], F32)
                nc.vector.tensor_copy(out=c_sb, in_=pc)
                pcb = psS.tile([128, 1], F32, tag="pss")
                nc.tensor.matmul(pcb, lhsT=ones_row, rhs=c_sb, start=True, stop=True)
                nc.vector.tensor_copy(out=v_cols[:, ND:ND + 1], in_=pcb)
            else:
                nc.vector.memset(v_cols, 0.0)

            # publish vc p-major: vc[p*9+j] <- v_cols[p, j]
            nc.gpsimd.dma_start(
                out=vc_in.ap().rearrange("(p j) -> p j", p=128), in_=v_cols
            )
            if use_coll:
                nc.gpsimd.collective_compute(
                    "AllGather", OP.bypass, replica_groups=RG,
                    ins=[vc_in.ap()], outs=[vc_all.ap()],
                )
            else:
                for r in range(NCORES):
                    nc.gpsimd.dma_start(out=vc_all[r, :], in_=vc_in.ap())

            if not do_phaseB:
                ydummy = spool.tile([1, NTOK], F32)
                nc.vector.memset(ydummy, 0.0)
                nc.gpsimd.dma_start(out=yout.ap(), in_=ydummy)
                continue

            # -------- Phase B: gate the full batch row on every core --------
            # lhsT [128, jd, 2E]: cols 0:E gate weights, E:2E expert v's
            wg_st = singles.tile([128, ND, E], F32)
            nc.sync.dma_start(
                out=wg_st, in_=wgl.ap().rearrange("p (j e) -> p j e", e=E)
            )
            wsb = singles.tile([128, ND, 2 * E], F32)
            nc.vector.tensor_copy(out=wsb[:, :, 0:E], in_=wg_st)
            wv_st = singles.tile([128, E, ND + 1], F32)
            nc.gpsimd.dma_start(
                out=wv_st,
                in_=vc_all.ap().rearrange("e (p j) -> p e j", p=128),
            )
            nc.vector.tensor_copy(
                out=wsb[:, :, E:2 * E],
                in_=wv_st[:, :, 0:ND].rearrange("p e j -> p j e"),
            )
            # c as a per-partition scalar on the [2E, NT] matmul output:
            # rows 0:E are logits (no bias), rows E:2E get c[e]
            c_col16 = singles.tile([2 * E, 1], F32)
            nc.vector.memset(c_col16[0:E, :], 0.0)
            nc.sync.dma_start(out=c_col16[E:2 * E, :], in_=vc_all[:, ND:ND + 1])

            xts = []
            for jd in range(ND):
                xTt = xpool.tile([128, T], F32)
                nc.sync.dma_start(out=xTt, in_=xTr[jd * 128:(jd + 1) * 128, :])
                xts.append(xTt)

            Gsb = singles.tile([128, NC, 2 * E], F32)
            for tcix in range(NC):
                pso = psO.tile([2 * E, 128], F32)
                for jd in range(ND):
                    nc.tensor.matmul(
                        pso, lhsT=wsb[:, jd, :],
                        rhs=xts[jd][:, tcix * 128:(tcix + 1) * 128],
                        start=(jd == 0), stop=(jd == ND - 1),
                    )
                gi = gpool.tile([2 * E, 128], F32)
                nc.vector.tensor_scalar(
                    out=gi, in0=pso, scalar1=c_col16, scalar2=None, op0=OP.add
                )
                pst = psG.tile([128, 2 * E], F32)
                nc.tensor.transpose(pst, gi, identF[0:2 * E, 0:2 * E])
                nc.scalar.copy(out=Gsb[:, tcix, :], in_=pst)

            # -------- batched top-2 gating over all NC chunks --------
            Ls = Gsb[:, :, 0:E]
            S2 = Gsb[:, :, E:2 * E]
            m0 = gpool.tile([128, NC], F32)
            nc.vector.tensor_reduce(out=m0, in_=Ls, axis=AX.X, op=OP.max)
            mask0 = gpool.tile([128, NC, E], F32)
            for tcix in range(NC):
                nc.vector.tensor_scalar(
                    out=mask0[:, tcix, :], in0=Gsb[:, tcix, 0:E],
                    scalar1=m0[:, tcix:tcix + 1], scalar2=None, op0=OP.is_equal,
                )
            scr3 = gpool.tile([128, NC, E], F32)
            nc.vector.tensor_mul(out=scr3, in0=S2, in1=mask0)
            se0 = gpool.tile([128, NC], F32)
            nc.vector.tensor_reduce(out=se0, in_=scr3, axis=AX.X, op=OP.add)
            L1 = gpool.tile([128, NC, E], F32)
            nc.vector.scalar_tensor_tensor(
                out=L1, in0=mask0, scalar=-1e30, in1=Ls,
                op0=OP.mult, op1=OP.add,
            )
            m1 = gpool.tile([128, NC], F32)
            nc.vector.tensor_reduce(out=m1, in_=L1, axis=AX.X, op=OP.max)
            mask1 = gpool.tile([128, NC, E], F32)
            for tcix in range(NC):
                nc.vector.tensor_scalar(
                    out=mask1[:, tcix, :], in0=L1[:, tcix, :],
                    scalar1=m1[:, tcix:tcix + 1], scalar2=None, op0=OP.is_equal,
                )
            scr3b = gpool.tile([128, NC, E], F32)
            nc.vector.tensor_mul(out=scr3b, in0=S2, in1=mask1)
            se1 = gpool.tile([128, NC], F32)
            nc.vector.tensor_reduce(out=se1, in_=scr3b, axis=AX.X, op=OP.add)
            dm = spool.tile([128, NC], F32)
            nc.vector.tensor_sub(out=dm, in0=m0, in1=m1)
            w0 = spool.tile([128, NC], F32)
            nc.scalar.activation(out=w0, in_=dm, func=ACTF.Sigmoid)
            d01 = spool.tile([128, NC], F32)
            nc.vector.tensor_sub(out=d01, in0=se0, in1=se1)
            sc = spool.tile([128, NC], F32)
            nc.vector.tensor_mul(out=sc, in0=d01, in1=w0)
            s_cols = spool.tile([128, NC], F32)
            nc.vector.tensor_add(out=s_cols, in0=sc, in1=se1)

            # -------- SBUF-local log-softmax over the row --------
            mrowp = spool.tile([128, 1], F32)
            nc.vector.tensor_reduce(out=mrowp, in_=s_cols, axis=AX.X, op=OP.max)
            pm = psS.tile([1, 128], F32, tag="pss")
            nc.tensor.transpose(pm, mrowp, identF)
            mrow = spool.tile([1, 1], F32)
            nc.vector.tensor_reduce(out=mrow, in_=pm, axis=AX.X, op=OP.max)
            pnegm = psS.tile([128, 1], F32, tag="pss")
            nc.tensor.matmul(pnegm, lhsT=negones_row, rhs=mrow, start=True, stop=True)
            negm = spool.tile([128, 1], F32)
            nc.vector.tensor_copy(out=negm, in_=pnegm)
            e8 = spool.tile([128, NC], F32)
            zrow = spool.tile([128, 1], F32)
            nc.scalar.activation(
                out=e8, in_=s_cols, func=ACTF.Exp, bias=negm, scale=1.0,
                accum_out=zrow,
            )
            pz = psS.tile([1, 1], F32, tag="pss")
            nc.tensor.matmul(pz, lhsT=zrow, rhs=ones_col, start=True, stop=True)
            zsb = spool.tile([1, 1], F32)
            nc.vector.tensor_copy(out=zsb, in_=pz)
            lnz = spool.tile([1, 1], F32)
            nc.scalar.activation(out=lnz, in_=zsb, func=ACTF.Ln)
            lse = spool.tile([1, 1], F32)
            nc.vector.tensor_add(out=lse, in0=lnz, in1=mrow)
            plse = psS.tile([128, 1], F32, tag="pss")
            nc.tensor.matmul(plse, lhsT=ones_row, rhs=lse, start=True, stop=True)
            lse_b = spool.tile([128, 1], F32)
            nc.vector.tensor_copy(out=lse_b, in_=plse)
            ysb = spool.tile([128, NC], F32)
            nc.vector.tensor_scalar(
                out=ysb, in0=s_cols, scalar1=lse_b, scalar2=None, op0=OP.subtract
            )

            # -------- select own half of the row, write out --------
            hsb = spool.tile([1, 1], F32)
            nc.sync.dma_start(out=hsb, in_=hsel[:, :])
            phb = psS.tile([128, 1], F32, tag="pss")
            nc.tensor.matmul(phb, lhsT=ones_row, rhs=hsb, start=True, stop=True)
            hb = spool.tile([128, 1], F32)
            nc.vector.tensor_copy(out=hb, in_=phb)
            NH2 = NC // 2
            yd = spool.tile([128, NH2], F32)
            nc.vector.tensor_sub(out=yd, in0=ysb[:, NH2:NC], in1=ysb[:, 0:NH2])
            ydh = spool.tile([128, NH2], F32)
            nc.vector.tensor_scalar(
                out=ydh, in0=yd, scalar1=hb, scalar2=None, op0=OP.mult
            )
            yown = spool.tile([128, NH2], F32)
            nc.vector.tensor_add(out=yown, in0=ydh, in1=ysb[:, 0:NH2])
            nc.gpsimd.dma_start(
                out=yout.ap().rearrange("(j p) -> p j", p=128), in_=yown
            )

    _legalize_waits(nc)
    return nc


def get_nc(reps: int = 1, variant: str = "full") -> bass.Bass:
    key = f"nc{reps}_{variant}"
    if key not in _CACHE:
        _CACHE[key] = _build_nc(reps, variant)
    return _CACHE[key]


def make_in_maps(x, wg, w1, b1, w2, b2) -> list[dict]:
    x = np.ascontiguousarray(np.asarray(x, dtype=np.float32))
    wg = np.ascontiguousarray(np.asarray(wg, dtype=np.float32))
    w1 = np.ascontiguousarray(np.asarray(w1, dtype=np.float32))
    b1 = np.ascontiguousarray(np.asarray(b1, dtype=np.float32))
    w2 = np.ascontiguousarray(np.asarray(w2, dtype=np.float32))
    b2 = np.ascontiguousarray(np.asarray(b2, dtype=np.float32))
    # gate lhsT layout [128, jd, e], bf16
    wgl = np.ascontiguousarray(
        wg.reshape(ND, 128, E).transpose(1, 0, 2).reshape(128, ND * E)
    )
    # per-row transposed x, bf16 (cores 2b and 2b+1 share row b)
    xT_rows = [
        np.ascontiguousarray(x[b_].T) for b_ in range(B)
    ]
    in_maps = []
    for c in range(NCORES):
        in_maps.append({
            "w1e": np.ascontiguousarray(w1[c]).astype(NPBF16),
            "w2e": np.ascontiguousarray(w2[c]).astype(NPBF16),
            "b1c": np.ascontiguousarray(b1[c].reshape(NH, 128).T),
            "b2c": np.ascontiguousarray(b2[c].reshape(ND, 128).T),
            "xTr": xT_rows[c // 2],
            "wgl": wgl,
            "hsel": np.full((1, 1), float(c % 2), dtype=np.float32),
        })
    return in_maps


_RUNNER_CACHE = {}


def _make_runner(nc):
    """One jitted sharded executable per nc, reused across retries/calls.
    (run_bass_kernel_spmd jits a fresh closure per call, so every retry
    would compile+load another executable otherwise.)"""
    import jax
    from concourse import bass2jax
    from jax.sharding import Mesh, PartitionSpec, NamedSharding
    from jax.experimental.shard_map import shard_map

    bass2jax.install_neuronx_cc_hook()
    pname = nc.partition_id_tensor.name if nc.partition_id_tensor else None
    in_names, out_names, out_avals, zero_outs = [], [], [], []
    for alloc in nc.m.functions[0].allocations:
        if not isinstance(alloc, mybir.MemoryLocationSet):
            continue
        name = alloc.memorylocations[0].name
        if alloc.kind == "ExternalInput":
            if name != pname:
                in_names.append(name)
        elif alloc.kind == "ExternalOutput":
            shape = tuple(alloc.tensor_shape)
            dtype = mybir.dt.np(alloc.dtype)
            out_names.append(name)
            out_avals.append(jax.core.ShapedArray(shape, dtype))
            zero_outs.append(np.zeros(shape, dtype))
    all_in_names = in_names + out_names + ([pname] if pname else [])

    def _body(*args):
        operands = list(args)
        if pname is not None:
            operands.append(bass2jax.partition_id_tensor())
        outs = bass2jax._bass_exec_p.bind(
            *operands,
            out_avals=tuple(out_avals),
            in_names=tuple(all_in_names),
            out_names=tuple(out_names),
            lowering_input_output_aliases=(),
            sim_require_finite=False,
            sim_require_nnan=False,
            nc=nc,
        )
        return tuple(outs)

    devices = jax.devices()[:NCORES]
    mesh = Mesh(np.asarray(devices), ("core",))
    spec = (PartitionSpec("core"),) * (len(in_names) + len(out_names))
    fn = jax.jit(
        shard_map(_body, mesh=mesh, in_specs=spec,
                  out_specs=(PartitionSpec("core"),) * len(out_names),
                  check_rep=False),
        keep_unused=True,
    )
    sh = NamedSharding(mesh, PartitionSpec("core"))
    return fn, in_names, zero_outs, sh


def _run_once(nc, in_maps) -> np.ndarray:
    import jax
    key = id(nc)
    if key not in _RUNNER_CACHE:
        _RUNNER_CACHE[key] = _make_runner(nc)
    fn, in_names, zero_outs, sh = _RUNNER_CACHE[key]
    concat_in = [
        jax.device_put(
            np.concatenate([np.asarray(in_maps[c][nm]) for c in range(NCORES)], axis=0),
            sh,
        )
        for nm in in_names
    ]
    concat_zero = [
        jax.device_put(np.zeros((NCORES * z.shape[0], *z.shape[1:]), z.dtype), sh)
        for z in zero_outs
    ]
    out = fn(*concat_in, *concat_zero)
    jax.block_until_ready(out)
    return np.asarray(out[0]).reshape(B, T).astype(np.float32)


def _run_once_fallback(nc, in_maps) -> np.ndarray:
    res = run_bass_kernel_spmd(nc, in_maps, core_ids=list(range(NCORES)))
    ys = [np.asarray(res.results[c]["y"]).reshape(NTOK) for c in range(NCORES)]
    return np.concatenate(ys).reshape(B, T).astype(np.float32)


def _looks_valid(y: np.ndarray) -> bool:
    """Output rows are log-softmax results, so logsumexp(row) must be ~0 and
    everything finite. Catches transient device-state garbage."""
    if not np.all(np.isfinite(y)):
        return False
    m = y.max(axis=1, keepdims=True)
    lse = m + np.log(np.exp(y - m).sum(axis=1, keepdims=True))
    return bool(np.abs(lse).max() < 1e-2)


def kernel(x, wg, w1, b1, w2, b2) -> np.ndarray:
    nc = get_nc()
    in_maps = make_in_maps(x, wg, w1, b1, w2, b2)
    # The axon-relay device occasionally returns one transiently-corrupt
    # execution (stale engine state from a previous tenant). Correct runs are
    # bit-identical, so require a self-consistency-checked repeat.
    prev = None
    last = None
    for _attempt in range(5):
        try:
            y = _run_once(nc, in_maps)
        except Exception:
            y = _run_once_fallback(nc, in_maps)
        last = y
        if not _looks_valid(y):
            prev = None
            continue
        if prev is not None and np.array_equal(prev, y):
            return y
        prev = y
    return prev if prev is not None else last


if __name__ == "__main__":
    import jax
    import reference
    cpu = jax.devices("cpu")[0]
    with jax.default_device(cpu):
        inputs = reference.setup_inputs()
        expected = np.asarray(reference.reference(**inputs))
    inputs_np = {k: np.asarray(v) for k, v in inputs.items()}
    actual = kernel(**inputs_np)
    err = np.abs(actual - expected).max()
    scale = np.abs(expected).max()
    print("max abs err:", err, " scale:", scale)
    print("Relative error:", err / scale)
    print("PASS" if err / scale < 2e-2 else "FAIL")
